# revision 24
# baseline (speedup 1.0000x reference)
"""Trainium2 Bass kernel for nn_MACE (2-layer MACE-style GNN, scalar energy).

Strategy (8 NeuronCores, SPMD), v2:
  - Nodes greedy-permuted onto (core, window, lane) so each of the 8x49
    windows of 128 nodes receives a near-equal incoming-edge load; per-core
    pad slots (22) are confined to the last window.
  - Edges live on the core owning their destination; windows are ranked by
    lo-half load so shared per-position tile capacities cl/ch stay tight.
  - Full node-feature table replicated in DRAM (bf16); per-edge source rows
    fetched with two big dma_gathers per 4-window group (int16 indices,
    table split at row 25088).
  - einsum('eh,ehl->eh'): only sum_l rw[:,:,l] matters -> rw_w3 host-folded
    to [H,H]; its bias seeded into PSUM by K=1 matmuls.
  - Scatter (segment_sum) via one-hot matmuls accumulated in a per-group
    [128,4,128] PSUM tile; sel masks built on DVE from a lane-id stream.
  - Node update interleaved into the edge phase per group (f32r matmuls,
    node-major LN stats); sqrt/reciprocal once per layer (ACT-table swaps
    stay rare); LN applied node-major (per-partition scale/bias on ACT).
  - Radial basis once, two passes (sqrt pass, sin2pi pass with mod-based
    range reduction); staged in DRAM as bf16.
  - Updated features AllGathered (bf16) into the next layer's gather table.
  - Per-core scalar partials summed on host.
"""
import heapq
import math
import sys
from contextlib import ExitStack

import numpy as np
import ml_dtypes

sys.path.insert(0, "/opt/trn_rl_repo")

import concourse.bacc as bacc  # noqa: E402
import concourse.bass as bass  # noqa: E402
import concourse.mybir as mybir  # noqa: E402
import concourse.tile as tile  # noqa: E402
from concourse.bass_utils import run_bass_kernel_spmd  # noqa: E402

AF = mybir.ActivationFunctionType
OP = mybir.AluOpType

N = 50000
E = 800000
H = 128
NB = 8
LMAX = 2
L = 2
CUTOFF = 5.0
NCORES = 8
NPC = 6272                 # nodes per core; 8*6272 = 50176 >= N
NPAD = NCORES * NPC
NW = NPC // 128            # 49 windows per core
SPLIT = 25088              # feats table row split (int16 index limit)
PADV = 1000.0              # lane value for padded edge slots
GW = 4                     # windows per group
NPPC = N // NCORES         # real nodes per core (6250)
REAL_LAST = 128 - (NPC - NPPC)   # 106 real lanes in the pad window
CKT = 3                    # tiles per MLP chunk
RTC = 48                   # rbf chunk tiles (multiple of 16)

F32 = mybir.dt.float32
F32R = mybir.dt.float32r
BF16 = mybir.dt.bfloat16
I16 = mybir.dt.int16

bf16 = ml_dtypes.bfloat16

SIM_SILU = False   # CoreSim lacks the Silu LUT; emulate via Sigmoid + mult

_CACHE = {}


# ---------------------------------------------------------------- host prep
def _balance_nodes(row):
    """Assign nodes to (core, window, lane): greedy balance of incoming-edge
    load over 8*49 windows; the last window of each core holds only 106 real
    nodes (lanes 106..127 stay pads)."""
    deg = np.bincount(row, minlength=N).astype(np.int64)
    order = np.argsort(-deg, kind="stable")
    nbins = NCORES * NW
    cap = np.full(nbins, 128, np.int64)
    cap[NW - 1::NW] = REAL_LAST
    load = np.zeros(nbins, np.int64)
    cnt = np.zeros(nbins, np.int64)
    heap = [(0, b) for b in range(nbins)]
    heapq.heapify(heap)
    node_bin = np.empty(N, np.int64)
    node_lane = np.empty(N, np.int64)
    for nd in order:
        while True:
            ld, b = heapq.heappop(heap)
            if cnt[b] < cap[b]:
                break
        node_bin[nd] = b
        node_lane[nd] = cnt[b]
        cnt[b] += 1
        load[b] += deg[nd]
        if cnt[b] < cap[b]:
            heapq.heappush(heap, (load[b], b))
    return node_bin, node_lane


def _prep(inputs):
    row, col = np.asarray(inputs["edge_index"], np.int64)
    pos = np.asarray(inputs["pos"], np.float32)
    an = np.asarray(inputs["atomic_numbers"], np.int64)

    rw_w3 = np.asarray(inputs["rw_w3"], np.float32)      # [L, H, 3H]
    rw_b3 = np.asarray(inputs["rw_b3"], np.float32)      # [L, 3H]
    w3eff = rw_w3.reshape(L, H, H, LMAX + 1).sum(-1)     # [L, H, H]
    b3eff = rw_b3.reshape(L, H, LMAX + 1).sum(-1)        # [L, H]

    node_bin, node_lane = _balance_nodes(row)
    node_core = node_bin // NW
    node_win = node_bin % NW

    # rank windows per core by lo-half load; pad window pinned last
    col_is_lo = node_core[col] < (NCORES // 2)
    binidx = node_bin[row]
    nlo_b = np.bincount(binidx[col_is_lo],
                        minlength=NCORES * NW).reshape(NCORES, NW)
    nhi_b = np.bincount(binidx[~col_is_lo],
                        minlength=NCORES * NW).reshape(NCORES, NW)

    pos_of_win = np.empty((NCORES, NW), np.int64)
    nlo_p = np.empty((NCORES, NW), np.int64)
    nhi_p = np.empty((NCORES, NW), np.int64)
    for c in range(NCORES):
        order = np.argsort(-nlo_b[c, :NW - 1], kind="stable")
        order = np.concatenate([order, [NW - 1]])
        pos_of_win[c, order] = np.arange(NW)
        nlo_p[c] = nlo_b[c, order]
        nhi_p[c] = nhi_b[c, order]

    cl = np.maximum(1, -(-nlo_p.max(axis=0) // 128))     # [NW] tiles
    ch = np.maximum(1, -(-nhi_p.max(axis=0) // 128))

    groups = [list(range(g0, min(g0 + GW, NW))) for g0 in range(0, NW, GW)]

    tile_info = []      # per global tile: (group, wof, is_first, is_last)
    g_meta = []
    t_cur = lo_cur = hi_cur = 0
    for ps in groups:
        GL = int(sum(cl[p] for p in ps))
        GH = int(sum(ch[p] for p in ps))
        seq = []
        for wi, p in enumerate(ps):
            seq += [(wi, 0)] * int(cl[p])
        for wi, p in enumerate(ps):
            seq += [(wi, 1)] * int(ch[p])
        first = {}
        last = {}
        for j, (wi, half) in enumerate(seq):
            if wi not in first:
                first[wi] = j
            last[wi] = j
        for j, (wi, half) in enumerate(seq):
            tile_info.append((len(g_meta), wi, j == first[wi],
                              j == last[wi]))
        g_meta.append(dict(ps=ps, GL=GL, GH=GH, t0=t_cur,
                           lo0=lo_cur, hi0=hi_cur))
        t_cur += GL + GH
        lo_cur += GL
        hi_cur += GH
    Tt, TLO, THI = t_cur, lo_cur, hi_cur
    Ttp = -(-Tt // RTC) * RTC
    EPC = Ttp * 128

    meta = dict(cl=tuple(int(x) for x in cl), ch=tuple(int(x) for x in ch),
                Tt=Tt, Ttp=Ttp, TLO=TLO, THI=THI, EPC=EPC)

    # global node slots & tables
    gslot = (node_core * NPC + pos_of_win[node_core, node_win] * 128
             + node_lane)
    feats0 = np.zeros((NPAD, H), np.float32)
    feats0[gslot] = np.asarray(inputs["node_emb"], np.float32)[an]
    ae = np.zeros(NPAD, np.float32)
    ae[gslot] = np.asarray(inputs["ae_emb"], np.float32)[an][:, 0]
    table0 = feats0.astype(bf16)

    e_core = node_core[row]
    e_pos = pos_of_win[node_core[row], node_win[row]]
    e_lane = node_lane[row]
    e_cslot = gslot[col]
    e_lo = e_cslot < SPLIT

    def wrap16(ix):
        a = ix.astype(np.int16).reshape(-1, 16).T
        return np.ascontiguousarray(np.tile(a, (8, 1)))

    def em(x, dt=np.float32):
        x = np.asarray(x, dt)
        tcnt = x.shape[0] // 128
        return np.ascontiguousarray(
            x.reshape(tcnt, 128, *x.shape[1:]).transpose(
                1, 0, *range(2, x.ndim + 1)))

    in_maps = []
    for c in range(NCORES):
        m = e_core == c
        ep, el, ecs, elo = e_pos[m], e_lane[m], e_cslot[m], e_lo[m]
        erow, ecol = row[m], col[m]

        pos_r = np.zeros((Tt * 128, 3), np.float32)
        pos_c = np.zeros((Tt * 128, 3), np.float32)
        relr = np.full(Tt * 128, PADV, np.float32)
        idx_lo = np.zeros(TLO * 128, np.int64)
        idx_hi = np.zeros(THI * 128, np.int64)

        for gm in g_meta:
            base_lo = gm["t0"] * 128
            base_hi = (gm["t0"] + gm["GL"]) * 128
            glo0 = gm["lo0"] * 128
            ghi0 = gm["hi0"] * 128
            off_lo = off_hi = 0
            for p in gm["ps"]:
                for half in (0, 1):
                    sel_m = (ep == p) & (elo if half == 0 else ~elo)
                    idx = np.nonzero(sel_m)[0]
                    idx = idx[np.argsort(ecs[idx], kind="stable")]
                    n = len(idx)
                    capn = int((cl if half == 0 else ch)[p]) * 128
                    assert n <= capn, (c, p, half, n, capn)
                    if half == 0:
                        s0 = base_lo + off_lo
                        idx_lo[glo0 + off_lo:glo0 + off_lo + n] = ecs[idx]
                        off_lo += capn
                    else:
                        s0 = base_hi + off_hi
                        idx_hi[ghi0 + off_hi:ghi0 + off_hi + n] = \
                            ecs[idx] - SPLIT
                        off_hi += capn
                    sl = slice(s0, s0 + n)
                    pos_r[sl] = pos[erow[idx]]
                    pos_c[sl] = pos[ecol[idx]]
                    relr[sl] = el[idx]

        pos_r = np.concatenate(
            [pos_r, np.zeros(((Ttp - Tt) * 128, 3), np.float32)])
        pos_c = np.concatenate(
            [pos_c, np.zeros(((Ttp - Tt) * 128, 3), np.float32)])

        nsl = c * NPC
        im = {
            "posr": em(pos_r), "posc": em(pos_c),
            "relr": em(relr, bf16).reshape(128, Tt, 1).copy(),
            "idx_lo": wrap16(idx_lo), "idx_hi": wrap16(idx_hi),
            "table0": table0,
            "feats_fm0": np.ascontiguousarray(feats0[nsl:nsl + NPC].T),
            "ae_nm": np.ascontiguousarray(
                ae[nsl:nsl + NPC].reshape(NW, 128).T),
        }
        in_maps.append(im)

    # packed constants
    pf_parts = {}
    pb_parts = {}

    def addf(name, arr):
        arr = np.asarray(arr, np.float32)
        a = np.zeros((128, arr.shape[1]), np.float32)
        a[:arr.shape[0]] = arr
        pf_parts[name] = a

    def addb(name, arr):
        arr = np.asarray(arr, np.float32).astype(bf16)
        a = np.zeros((128, arr.shape[1]), bf16)
        a[:arr.shape[0]] = arr
        pb_parts[name] = a

    addf("cz", np.zeros((128, 1), np.float32))
    addf("cmpi", np.full((128, 1), -math.pi, np.float32))
    addf("eps", np.full((128, 1), 1e-5, np.float32))
    addf("ones_mean", np.full((128, 1), 1.0 / H, np.float32))
    addf("onescol", np.ones((128, 1), np.float32))
    addf("one11", np.ones((1, 1), np.float32))
    addf("ones1r", np.ones((1, 512), np.float32))
    addf("idenf", np.eye(128, dtype=np.float32))
    addf("kfrac", np.tile(np.arange(1, NB + 1, dtype=np.float32)
                          * math.pi / CUTOFF, (128, 1)))
    valid = np.ones((128, 128), np.float32)
    valid[:, REAL_LAST:] = 0.0
    addf("valid", valid)
    for l in range(L):
        addf(f"b1_{l}", np.asarray(inputs["rw_b1"][l],
                                   np.float32).reshape(128, 1))
        addf(f"b2_{l}", np.asarray(inputs["rw_b2"][l],
                                   np.float32).reshape(128, 1))
        addf(f"linA_{l}", np.asarray(inputs["lin_w"][l][:H], np.float32))
        addf(f"linB_{l}", np.asarray(inputs["lin_w"][l][H:], np.float32))
        addf(f"linb_{l}", np.asarray(inputs["lin_b"][l],
                                     np.float32).reshape(1, 128))
        addf(f"lngbc_{l}", np.tile(np.asarray(inputs["ln_g"][l], np.float32)
                                   .reshape(1, 128), (128, 1)))
        addf(f"lnbbc_{l}", np.tile(np.asarray(inputs["ln_b"][l], np.float32)
                                   .reshape(1, 128), (128, 1)))
    addf("row1", np.asarray(inputs["ro_w1"], np.float32))
    addf("rob1", np.asarray(inputs["ro_b1"], np.float32).reshape(128, 1))
    addf("row2", np.asarray(inputs["ro_w2"], np.float32).reshape(128, 1))

    addb("iden", np.eye(128, dtype=np.float32))
    addb("iota", np.tile(np.arange(128, dtype=np.float32), (128, 1)))
    addb("ones1", np.ones((1, 128), np.float32))
    for l in range(L):
        addb(f"w1_{l}", np.asarray(inputs["rw_w1"][l], np.float32))
        addb(f"w2_{l}", np.asarray(inputs["rw_w2"][l], np.float32))
        addb(f"w3_{l}", w3eff[l])
        addb(f"b3_{l}", b3eff[l].reshape(1, 128))

    def pack(parts):
        offs = {}
        o = 0
        for k, v in parts.items():
            offs[k] = (o, v.shape[1])
            o += v.shape[1]
        return np.concatenate(list(parts.values()), axis=1), offs

    pfbuf, pfoff = pack(pf_parts)
    pbbuf, pboff = pack(pb_parts)
    for im in in_maps:
        im["packf"] = pfbuf
        im["packb"] = pbbuf

    layout = dict(g_meta=g_meta, tile_info=tile_info, pfoff=pfoff,
                  pboff=pboff, pfw=pfbuf.shape[1], pbw=pbbuf.shape[1])
    host = dict(
        ro_b2=float(np.asarray(inputs["ro_b2"]).reshape(-1)[0]),
        scale=float(np.asarray(inputs["scale"])),
        shift=float(np.asarray(inputs["shift"])),
    )
    return in_maps, meta, layout, host


# ---------------------------------------------------------------- program
def _build(meta, layout):
    Tt, Ttp, EPC = meta["Tt"], meta["Ttp"], meta["EPC"]
    TLO, THI = meta["TLO"], meta["THI"]
    g_meta = layout["g_meta"]
    tile_info = layout["tile_info"]
    pfoff, pboff = layout["pfoff"], layout["pboff"]
    NG = len(g_meta)
    GTmax = max(gm["GL"] + gm["GH"] for gm in g_meta)
    GHLmax = max(max(gm["GL"], gm["GH"]) for gm in g_meta)

    nc = bacc.Bacc("TRN2", target_bir_lowering=False, debug=False,
                   num_devices=NCORES, num_swdge_queues=4,
                   dynamic_dma_scratch_size=2 ** 15)

    def din(name, shape, dt=F32):
        return nc.dram_tensor(name, shape, dt, kind="ExternalInput")

    posr = din("posr", [128, Ttp, 3])
    posc = din("posc", [128, Ttp, 3])
    relr_d = din("relr", [128, Tt, 1], BF16)
    idx_lo = din("idx_lo", [128, TLO * 8], I16)
    idx_hi = din("idx_hi", [128, THI * 8], I16)
    table0 = din("table0", [NPAD, H], BF16)
    feats_fm0 = din("feats_fm0", [H, NPC])
    ae_nm_d = din("ae_nm", [128, NW])
    packf = din("packf", [128, layout["pfw"]])
    packb = din("packb", [128, layout["pbw"]], BF16)

    out = nc.dram_tensor("out", [1, 1], F32, kind="ExternalOutput")

    with tile.TileContext(nc) as tc, ExitStack() as ctx:
        dram = ctx.enter_context(tc.tile_pool(name="dram", bufs=1,
                                              space="DRAM"))
        rbf_dram = dram.tile([NB, EPC], BF16)
        ag_in = dram.tile([NPC, H], BF16)
        table1 = nc.dram_tensor("table1", [NPAD, H], BF16,
                                addr_space="Shared")

        cpool = ctx.enter_context(tc.tile_pool(name="consts", bufs=1))
        pf = cpool.tile([128, layout["pfw"]], F32, tag="pf")
        pb = cpool.tile([128, layout["pbw"]], BF16, tag="pb")
        nc.sync.dma_start(pf[:], packf.ap())
        nc.sync.dma_start(pb[:], packb.ap())

        def F(name, rows=128):
            o, w = pfoff[name]
            return pf[0:rows, o:o + w]

        def B(name, rows=128):
            o, w = pboff[name]
            return pb[0:rows, o:o + w]

        nc.const_aps.aps[(F32, 0.0)] = F("cz")[:, 0:1]
        nc.const_aps.aps[(F32, -math.pi)] = F("cmpi")[:, 0:1]

        relr_s = cpool.tile([128, Tt, 1], BF16, tag="relr")
        nc.sync.dma_start(relr_s[:], relr_d.ap())
        idxl_s = cpool.tile([128, TLO * 8], I16, tag="idxl")
        nc.sync.dma_start(idxl_s[:], idx_lo.ap())
        idxh_s = cpool.tile([128, THI * 8], I16, tag="idxh")
        nc.sync.dma_start(idxh_s[:], idx_hi.ap())
        ae_s = cpool.tile([128, NW], F32, tag="ae")
        nc.sync.dma_start(ae_s[:], ae_nm_d.ap())

        feats_fm = cpool.tile([H, NPC], F32, tag="feats_fm")
        nc.sync.dma_start(feats_fm[:], feats_fm0.ap())
        agg = cpool.tile([H, NPC], F32, tag="agg")
        dd_all = cpool.tile([128, Ttp], F32, tag="dd_all")
        er_s = cpool.tile([1, 512], F32, tag="er_s")

        # PSUM pools: pA/pB/pC rotate (2 banks each), wps + st pinned.
        pp = ctx.enter_context(tc.tile_pool(name="pp", bufs=2, space="PSUM"))
        wpp = ctx.enter_context(tc.tile_pool(name="wpp", bufs=1,
                                             space="PSUM"))
        stp = ctx.enter_context(tc.tile_pool(name="stp", bufs=1,
                                             space="PSUM"))

        # ---------------- RBF phase (two passes) ----------------
        NCH = Ttp // RTC
        rp = ctx.enter_context(tc.tile_pool(name="rp", bufs=2))
        for ci in range(NCH):
            t0 = ci * RTC
            pr = rp.tile([128, RTC, 3], F32, tag="pr")
            pc = rp.tile([128, RTC, 3], F32, tag="pc")
            nc.sync.dma_start(pr[:], posr.ap()[:, t0:t0 + RTC, :])
            nc.sync.dma_start(pc[:], posc.ap()[:, t0:t0 + RTC, :])
            dx = rp.tile([128, RTC, 3], F32, tag="dx")
            nc.vector.tensor_tensor(out=dx[:], in0=pc[:], in1=pr[:],
                                    op=OP.subtract)
            nc.vector.tensor_tensor(out=dx[:], in0=dx[:], in1=dx[:],
                                    op=OP.mult)
            d2 = rp.tile([128, RTC], F32, tag="d2")
            nc.vector.tensor_reduce(out=d2[:], in_=dx[:],
                                    axis=mybir.AxisListType.X, op=OP.add)
            nc.scalar.activation(dd_all[:, t0:t0 + RTC], d2[:], AF.Sqrt)
        for ci in range(NCH):
            t0 = ci * RTC
            dd = dd_all[:, t0:t0 + RTC]
            # negated env: co = (sin(d*pi/(2C))^2 - 1) * (d < C) / max(d,1e-3)
            # rbf_k = -sin(2*pi*frac(d*k/(2C)) - pi) * env/d; signs cancel.
            co = rp.tile([128, RTC], F32, tag="co")
            nc.scalar.activation(co[:], dd, AF.Sin,
                                 scale=math.pi / (2 * CUTOFF))
            nc.scalar.activation(co[:], co[:], AF.Square)
            msk = rp.tile([128, RTC], F32, tag="msk")
            nc.vector.tensor_scalar(out=msk[:], in0=dd,
                                    scalar1=float(CUTOFF), scalar2=None,
                                    op0=OP.is_lt)
            nc.vector.tensor_tensor(out=co[:], in0=co[:], in1=msk[:],
                                    op=OP.mult)
            nc.vector.tensor_tensor(out=co[:], in0=co[:], in1=msk[:],
                                    op=OP.subtract)
            dcl = rp.tile([128, RTC], F32, tag="dcl")
            nc.vector.tensor_scalar(out=dcl[:], in0=dd, scalar1=1e-3,
                                    scalar2=None, op0=OP.max)
            rec = rp.tile([128, RTC], F32, tag="rec")
            nc.vector.reciprocal(rec[:], dcl[:])
            nc.vector.tensor_tensor(out=co[:], in0=co[:], in1=rec[:],
                                    op=OP.mult)
            xk = rp.tile([128, RTC, NB], F32, tag="xk")
            nc.vector.tensor_tensor(
                out=xk[:],
                in0=dd[:, :, None].to_broadcast([128, RTC, NB]),
                in1=F("kfrac")[:, None, :].to_broadcast([128, RTC, NB]),
                op=OP.mult)
            sub = rp.tile([128, RTC, NB], F32, tag="sub")
            for cc in (8 * math.pi, 4 * math.pi, 2 * math.pi):
                nc.vector.tensor_scalar(out=sub[:], in0=xk[:],
                                        scalar1=float(cc), scalar2=float(cc),
                                        op0=OP.is_ge, op1=OP.mult)
                nc.vector.tensor_tensor(out=xk[:], in0=xk[:], in1=sub[:],
                                        op=OP.subtract)
            tau_lo = float(np.nextafter(np.float32(2 * math.pi),
                                        np.float32(0)))
            nc.vector.tensor_scalar(out=xk[:], in0=xk[:], scalar1=tau_lo,
                                    scalar2=None, op0=OP.min)
            sn = rp.tile([128, RTC, NB], F32, tag="sub")
            nc.scalar.activation(sn[:], xk[:], AF.Sin, bias=-math.pi)
            rbb = rp.tile([128, RTC, NB], BF16, tag="rbb")
            nc.vector.tensor_tensor(
                out=rbb[:], in0=sn[:],
                in1=co[:, :, None].to_broadcast([128, RTC, NB]),
                op=OP.mult)
            for g16 in range(RTC // 16):
                tg = t0 + 16 * g16
                tp_ = pp.tile([128, 128], BF16, tag="pA")
                nc.tensor.transpose(tp_[:], rbb[:, 16 * g16:16 * g16 + 16, :],
                                    B("iden"))
                tsb = rp.tile([128, 128], BF16, tag="tsb")
                nc.vector.tensor_copy(out=tsb[:], in_=tp_[:])
                base = rbf_dram[:]
                dram_ap = bass.AP(base.tensor, base.offset + 128 * tg,
                                  [[128, 16], [EPC, NB], [1, 128]])
                nc.sync.dma_start(dram_ap, tsb[:])

        # ---------------- main pools ----------------
        gp = ctx.enter_context(tc.tile_pool(name="gath",
                                            bufs=1 if SIM_SILU else 2))
        rbp = ctx.enter_context(tc.tile_pool(name="rbp", bufs=2))
        mp = ctx.enter_context(tc.tile_pool(name="mp", bufs=3))
        npo = ctx.enter_context(tc.tile_pool(name="npo", bufs=2))

        def silu(out_t, in_ap, bias_ap):
            if not SIM_SILU:
                nc.scalar.activation(out_t, in_ap, AF.Silu, bias=bias_ap)
            else:
                shp = list(in_ap.shape)
                xt = mp.tile(shp, F32, tag="sim_x", bufs=1)
                nc.scalar.activation(xt[:], in_ap, AF.Identity, bias=bias_ap)
                sg = mp.tile(shp, F32, tag="sim_s", bufs=1)
                nc.scalar.activation(sg[:], xt[:], AF.Sigmoid)
                nc.vector.tensor_tensor(out=out_t, in0=xt[:], in1=sg[:],
                                        op=OP.mult)

        self_q = [0]

        def layer(l, tab_lo, tab_hi):
            st = stp.tile([128, 128], F32, tag="st")
            # ---------------- edge phase ----------------
            for gi, gm in enumerate(g_meta):
                GL, GH, t0 = gm["GL"], gm["GH"], gm["t0"]
                GT = GL + GH
                nw_g = len(gm["ps"])
                nfree = 128 * nw_g
                gsl = slice(128 * GW * gi, 128 * GW * gi + nfree)
                nj = gp.tile([128, GTmax, 128], BF16, tag="nj")
                GMAX = 8           # <= 1024 idxs per gather (HW SWDGE ring)
                for j0 in range(0, GL, GMAX):
                    jn = min(GMAX, GL - j0)
                    nc.gpsimd.dma_gather(
                        nj[:, j0:j0 + jn, :], tab_lo,
                        idxl_s[:, (gm["lo0"] + j0) * 8:
                               (gm["lo0"] + j0 + jn) * 8],
                        jn * 128, jn * 128, H, single_packet=True,
                        queue_num=self_q[0] % 4)
                    self_q[0] += 1
                for j0 in range(0, GH, GMAX):
                    jn = min(GMAX, GH - j0)
                    nc.gpsimd.dma_gather(
                        nj[:, GL + j0:GL + j0 + jn, :], tab_hi,
                        idxh_s[:, (gm["hi0"] + j0) * 8:
                               (gm["hi0"] + j0 + jn) * 8],
                        jn * 128, jn * 128, H, single_packet=True,
                        queue_num=self_q[0] % 4)
                    self_q[0] += 1
                rbfg = rbp.tile([NB, GHLmax * 128], BF16, tag="rbfg")
                nc.sync.dma_start(rbfg[:, 0:GL * 128],
                                  rbf_dram[:, t0 * 128:(t0 + GL) * 128])
                rbfg2 = rbp.tile([NB, GHLmax * 128], BF16, tag="rbfg2")
                nc.sync.dma_start(
                    rbfg2[:, 0:GH * 128],
                    rbf_dram[:, (t0 + GL) * 128:(t0 + GT) * 128])

                wps = wpp.tile([128, GW, 128], F32, tag="wps")

                # process each window's lo+hi tiles consecutively so its
                # PSUM accumulation group closes before the next one opens
                chunks = []
                lo_off = 0
                hi_off = GL
                for wi, p in enumerate(gm["ps"]):
                    ncl = int(meta["cl"][p])
                    nch = int(meta["ch"][p])
                    wtiles = []
                    for j in range(0, ncl, CKT):
                        wtiles.append((lo_off + j, min(CKT, ncl - j),
                                       rbfg, 0))
                    for j in range(0, nch, CKT):
                        wtiles.append((hi_off + j, min(CKT, nch - j),
                                       rbfg2, GL))
                    lo_off += ncl
                    hi_off += nch
                    chunks.append((wi, wtiles))
                for wi, wtiles in chunks:
                    nt_w = sum(ck for _, ck, _, _ in wtiles)
                    done_w = 0
                    for (c0, ck, rbsrc, rb0) in wtiles:
                        h1p = pp.tile([128, CKT, 128], F32, tag="pA")
                        nc.tensor.matmul(
                            h1p[:, 0:ck, :], lhsT=B(f"w1_{l}", rows=NB),
                            rhs=rbsrc[:, (c0 - rb0) * 128:
                                      (c0 - rb0 + ck) * 128],
                            start=True, stop=True)
                        h1 = mp.tile([128, CKT, 128], BF16, tag="h1")
                        silu(h1[:, 0:ck, :], h1p[:, 0:ck, :],
                             F(f"b1_{l}")[:, 0:1])
                        h2p = pp.tile([128, CKT, 128], F32, tag="pB")
                        nc.tensor.matmul(h2p[:, 0:ck, :], lhsT=B(f"w2_{l}"),
                                         rhs=h1[:, 0:ck, :].opt(),
                                         start=True, stop=True)
                        h2 = mp.tile([128, CKT, 128], BF16, tag="h2")
                        silu(h2[:, 0:ck, :], h2p[:, 0:ck, :],
                             F(f"b2_{l}")[:, 0:1])
                        rwp = pp.tile([128, CKT, 128], F32, tag="pC")
                        for k in range(ck):
                            nc.tensor.matmul(rwp[:, k, :],
                                             lhsT=B("ones1", rows=1),
                                             rhs=B(f"b3_{l}", rows=1),
                                             start=True, stop=False)
                            nc.tensor.matmul(rwp[:, k, :], lhsT=h2[:, k, :],
                                             rhs=B(f"w3_{l}"),
                                             start=False, stop=True)
                        sel = mp.tile([128, CKT, 128], BF16, tag="sel")
                        nc.vector.tensor_tensor(
                            out=sel[:, 0:ck, :],
                            in0=relr_s[:, t0 + c0:t0 + c0 + ck, :]
                            .to_broadcast([128, ck, 128]),
                            in1=B("iota")[:, None, :]
                            .to_broadcast([128, ck, 128]), op=OP.is_equal)
                        msgs = mp.tile([128, CKT, 128], BF16, tag="msgs")
                        nc.vector.tensor_tensor(
                            out=msgs[:, 0:ck, :], in0=nj[:, c0:c0 + ck, :],
                            in1=rwp[:, 0:ck, :], op=OP.mult)
                        for k in range(ck):
                            nc.tensor.matmul(wps[:, wi, :],
                                             lhsT=msgs[:, k, :],
                                             rhs=sel[:, k, :],
                                             start=done_w + k == 0,
                                             stop=done_w + k == nt_w - 1)
                        done_w += ck
                nc.scalar.activation(agg[:, gsl], wps[:, 0:nw_g, :].opt(),
                                     AF.Copy)

                # interleaved node stats for this group
                up = pp.tile([128, 512], F32, tag="pA")
                nc.tensor.matmul(up[:, 0:nfree],
                                 lhsT=F(f"linA_{l}"),
                                 rhs=feats_fm[:, gsl],
                                 start=True, stop=False)
                nc.tensor.matmul(up[:, 0:nfree],
                                 lhsT=F(f"linB_{l}"),
                                 rhs=agg[:, gsl],
                                 start=False, stop=False)
                nc.tensor.matmul(up[:, 0:nfree],
                                 lhsT=F(f"linb_{l}", rows=1),
                                 rhs=F("ones1r", rows=1)[:, 0:nfree]
                                 ,
                                 start=False, stop=True)
                nc.vector.tensor_tensor(out=agg[:, gsl], in0=up[:, 0:nfree],
                                        in1=feats_fm[:, gsl], op=OP.add)
                x2 = npo.tile([128, 512], F32, tag="x2")
                nc.scalar.activation(x2[:, 0:nfree], agg[:, gsl], AF.Square)
                for wi in range(nw_g):
                    p = GW * gi + wi
                    wsl = slice(128 * p, 128 * (p + 1))
                    nc.tensor.matmul(st[:, 2 * p:2 * p + 1],
                                     lhsT=agg[:, wsl],
                                     rhs=F("ones_mean")[:, 0:1],
                                     start=True, stop=True)
                    nc.tensor.matmul(st[:, 2 * p + 1:2 * p + 2],
                                     lhsT=x2[:, 128 * wi:128 * (wi + 1)],
                                     rhs=F("ones_mean")[:, 0:1],
                                     start=True, stop=True)

            # ---------------- LN batch (once per layer) ----------------
            sts = npo.tile([128, 2 * NW], F32, tag="sts")
            nc.vector.tensor_copy(out=sts[:], in_=st[:, 0:2 * NW])
            mean_v = sts[:, 0:2 * NW:2]
            e2_v = sts[:, 1:2 * NW:2]
            vv = npo.tile([128, NW], F32, tag="vv")
            nc.vector.tensor_tensor(out=vv[:], in0=mean_v, in1=mean_v,
                                    op=OP.mult)
            nc.vector.tensor_tensor(out=vv[:], in0=e2_v, in1=vv[:],
                                    op=OP.subtract)
            sd = npo.tile([128, NW], F32, tag="sd")
            nc.scalar.activation(sd[:], vv[:], AF.Sqrt,
                                 bias=F("eps")[:, 0:1])
            rs = npo.tile([128, NW], F32, tag="rs", bufs=2)
            nc.vector.reciprocal(rs[:], sd[:])
            nbn = npo.tile([128, NW], F32, tag="nbn", bufs=2)
            nc.vector.tensor_tensor(out=nbn[:], in0=rs[:], in1=mean_v,
                                    op=OP.mult)
            nc.vector.tensor_scalar(out=nbn[:], in0=nbn[:], scalar1=-1.0,
                                    scalar2=None, op0=OP.mult)

            # -------- apply (node-major) + (ag_in | readout) --------
            for gi, gm in enumerate(g_meta):
                nw_g = len(gm["ps"])
                nfree = 128 * nw_g
                gsl = slice(128 * GW * gi, 128 * GW * gi + nfree)
                xtp = pp.tile([128, GW, 128], F32, tag="pC")
                for wi in range(nw_g):
                    nc.tensor.transpose(
                        xtp[:, wi, :],
                        agg[:, 128 * (GW * gi + wi):128 * (GW * gi + wi + 1)],
                        F("idenf"))
                ynm = npo.tile([128, GW, 128], F32, tag="t2")
                for wi in range(nw_g):
                    p = GW * gi + wi
                    nc.scalar.activation(ynm[:, wi, :], xtp[:, wi, :],
                                         AF.Identity,
                                         bias=nbn[:, p:p + 1],
                                         scale=rs[:, p:p + 1])
                nc.vector.tensor_tensor(
                    out=ynm[:, 0:nw_g, :], in0=ynm[:, 0:nw_g, :],
                    in1=F(f"lngbc_{l}")[:, None, :]
                    .to_broadcast([128, nw_g, 128]), op=OP.mult)
                nc.vector.tensor_tensor(
                    out=ynm[:, 0:nw_g, :], in0=ynm[:, 0:nw_g, :],
                    in1=F(f"lnbbc_{l}")[:, None, :]
                    .to_broadcast([128, nw_g, 128]), op=OP.add)
                if l == 0:
                    yb = npo.tile([128, GW, 128], BF16, tag="yb")
                    nc.vector.tensor_copy(out=yb[:, 0:nw_g, :],
                                          in_=ynm[:, 0:nw_g, :])
                    nc.sync.dma_start(
                        ag_in[128 * GW * gi:128 * GW * gi + nfree, :],
                        yb[:, 0:nw_g, :].opt())
                # transpose back to feature-major
                ybp = pp.tile([128, GW, 128], F32, tag="pA")
                for wi in range(nw_g):
                    nc.tensor.transpose(ybp[:, wi, :], ynm[:, wi, :],
                                        F("idenf"))
                nc.vector.tensor_copy(out=feats_fm[:, gsl],
                                      in_=ybp[:, 0:nw_g, :].opt())
                if l == 1:
                    ap_ = pp.tile([128, 512], F32, tag="pC")
                    nc.tensor.matmul(ap_[:, 0:nfree],
                                     lhsT=F("row1"),
                                     rhs=feats_fm[:, gsl],
                                     start=True, stop=True)
                    a = npo.tile([128, 512], F32, tag="a", bufs=1)
                    silu(a[:, 0:nfree], ap_[:, 0:nfree], F("rob1")[:, 0:1])
                    if gi == NG - 1:
                        nc.vector.tensor_tensor(out=a[:, 0:nfree],
                                                in0=a[:, 0:nfree],
                                                in1=F("valid")[:, 0:nfree],
                                                op=OP.mult)
                    ep = pp.tile([1, 512], F32, tag="pB")
                    nc.tensor.matmul(ep[:, 0:nfree],
                                     lhsT=F("row2"),
                                     rhs=a[:, 0:nfree],
                                     start=True, stop=True)
                    if gi == 0:
                        nc.vector.tensor_copy(out=er_s[:, 0:nfree],
                                              in_=ep[:, 0:nfree])
                    else:
                        nc.vector.tensor_tensor(out=er_s[:, 0:nfree],
                                                in0=er_s[:, 0:nfree],
                                                in1=ep[:, 0:nfree],
                                                op=OP.add)

        layer(0, table0.ap()[0:SPLIT, :], table0.ap()[SPLIT:NPAD, :])
        nc.gpsimd.collective_compute(
            "AllGather", OP.bypass,
            replica_groups=[list(range(NCORES))],
            ins=[ag_in.opt()], outs=[table1.ap().opt()])
        layer(1, table1.ap()[0:SPLIT, :], table1.ap()[SPLIT:NPAD, :])

        # ---------------- final reduction ----------------
        erd = cpool.tile([1, 1], F32, tag="erd")
        nc.vector.tensor_reduce(out=erd[:], in_=er_s[:],
                                axis=mybir.AxisListType.X, op=OP.add)
        aer = cpool.tile([128, 1], F32, tag="aer")
        nc.vector.tensor_reduce(out=aer[:], in_=ae_s[:],
                                axis=mybir.AxisListType.X, op=OP.add)
        tot = pp.tile([1, 1], F32, tag="pC")
        nc.tensor.matmul(tot[:], lhsT=aer[:], rhs=F("onescol")[:, 0:1],
                         start=True, stop=False)
        nc.tensor.matmul(tot[:], lhsT=erd[:],
                         rhs=F("one11", rows=1)[:, 0:1],
                         start=False, stop=True)
        tsb1 = cpool.tile([1, 1], F32, tag="tsb1")
        nc.vector.tensor_copy(out=tsb1[:], in_=tot[:])
        nc.sync.dma_start(out.ap(), tsb1[:])

    nc.compile()
    return nc


# ---------------------------------------------------------------- entry
def kernel(**inputs):
    in_maps, meta, layout, host = _prep(inputs)
    key = tuple(sorted(meta.items()))
    if key not in _CACHE:
        _CACHE[key] = _build(meta, layout)
    nc = _CACHE[key]
    res = run_bass_kernel_spmd(nc, in_maps, core_ids=list(range(NCORES)))
    partials = [float(r["out"][0, 0]) for r in res.results]
    total = sum(partials) + host["ro_b2"] * N
    return np.float32(total * host["scale"] + host["shift"])
